# revision 1
# baseline (speedup 1.0000x reference)
"""Bass/Trainium2 kernel for nn_BmmEnsemble (ensemble-of-MLPs atomic energy sum).

Sharding: 8 cores; core c owns species c//2, half c%2 (12500/2 = 6250 atoms).
Each core runs a 3-layer MLP (1008->256->192->160, CELU) for its species'
8 ensemble members on its 6250 atoms, producing per-feature sums of the
layer-3 activations. Layer 4 ([160]->[1]), the ensemble mean, the CELU
constant shifts and the final atom sum are all linear, so they are folded
into host-side fp64 math on the tiny reduced vectors.

Device layout: activations are [features(partitions) x atoms(free)], weights
are natural [din(partitions) x dout(free)] so NO on-device transposes are
needed; the host supplies aev pre-gathered and transposed per core (with a
ones row appended so the layer-1 bias rides the matmul).

CELU algebra (alpha=0.1):
  g(x) := celu(x) + alpha = max(x + alpha, min(alpha*e^(x/alpha), alpha))
  (exact: for x>=0 both relu branch and saturated exp-min give x+alpha;
   for x<0 the exp branch wins since alpha*e^(x/alpha) >= x+alpha).
  The "+alpha" shift folds into the next layer's bias:
  b_adj = b - alpha * colsum(W).
  Layer 1: z' = z + b1 + alpha from the matmul ones-row; then
    u' = ACT Exp(10*z' + (ln a - 1)) = a*e^(x/alpha);  g = DVE stt (u' min a) max z'.
  Layer 2: u' = ACT Exp(10*z + 10*b_adj + ln a);  r = DVE (z add b_adj) max 0;
    g = DVE stt (u' min a) add r   [g = relu + min form, same value].
  Layer 3 (only per-feature atom sums needed):
    Sum g3 = Sum relu(x3) + Sum min(u3', a):
    ACT Relu(z3 + b_adj) with accum_out  +  DVE (u3' min a) add 0 with accum_out.
  Host: h3 = g3 - alpha, then layer 4 / ensemble mean / b4 terms in fp64.

Emission is software-pipelined per slot t: PE runs [L1(t), L2(t-1), L3(t-2)]
so the PE queue never waits on the elementwise chain of the same iteration.
"""

import numpy as np

import concourse.bacc as bacc
import concourse.tile as tile
import concourse.mybir as mybir
from concourse.bass_utils import run_bass_kernel_spmd

F32 = mybir.dt.float32
F32R = mybir.dt.float32r
AF = mybir.ActivationFunctionType
ALU = mybir.AluOpType

S = 4
E = 8
N = 50000
AEV = 1008
ALPHA = 0.1
LN_A = float(np.log(ALPHA))
NCORES = 8
NA = N // S // 2           # atoms per core: 6250
CH = 512                   # atom chunk (matmul free dim)
NCH = (NA + CH - 1) // CH  # 13 chunks (12 x 512 + 106)
D1, D2, D3 = 256, 192, 160
K1 = 1024                  # aev (1008) + ones row (1) + zero pad to 8x128
KC1 = [(kc * 128, min(128, K1 - kc * 128)) for kc in range((K1 + 127) // 128)]  # 8 chunks
L2K = [(0, 128), (128, 128)]          # K chunks of 256
L2M = [(0, 128), (128, 64)]           # M chunks of 192
L3K = [(0, 128), (128, 128)]          # K chunks of 192 zero-padded to 256
L3M = [(0, 128), (128, 32)]           # M chunks of 160
NCOL = E * NCH * 2                    # accum columns: (e, chunk) x {relu, minexp}
NSLOT = E * NCH                       # 104 pipeline slots

USE_F32R = True


def _build():
    nc = bacc.Bacc("TRN2", target_bir_lowering=False, debug=False, num_devices=NCORES)

    MMDT = F32R if USE_F32R else F32
    xT = nc.dram_tensor("xT", [K1, NA], MMDT, kind="ExternalInput")
    w1 = nc.dram_tensor("w1", [E, K1, D1], MMDT, kind="ExternalInput")
    w2 = nc.dram_tensor("w2", [E, 256, D2], MMDT, kind="ExternalInput")
    w3 = nc.dram_tensor("w3", [E, 256, D3], MMDT, kind="ExternalInput")
    b2 = nc.dram_tensor("b2", [E, 128, 4], F32, kind="ExternalInput")
    b3 = nc.dram_tensor("b3", [E, 128, 4], F32, kind="ExternalInput")
    acc1 = nc.dram_tensor("acc1", [128, NCOL], F32, kind="ExternalOutput")
    acc2 = nc.dram_tensor("acc2", [32, NCOL], F32, kind="ExternalOutput")

    with tile.TileContext(nc) as tc:
        with (
            tc.tile_pool(name="wp", bufs=1) as wp,
            tc.tile_pool(name="xp", bufs=2) as xp,
            tc.tile_pool(name="gp", bufs=4) as gp,
            tc.tile_pool(name="sp", bufs=4) as sp,
            tc.tile_pool(name="ps", bufs=8, space="PSUM") as ps,
        ):
            # ---- x prefetch helper ----
            xtiles = {}   # ci -> list of tiles

            def emit_x_dma(ci):
                if ci >= NCH or ci in xtiles:
                    return
                off = ci * CH
                na = min(CH, NA - off)
                lst = []
                for kc, (k0, kw) in enumerate(KC1):
                    t = xp.tile([kw, CH], MMDT, tag=f"x_{kc}")
                    nc.sync.dma_start(t[:, :na], xT[k0:k0 + kw, off:off + na])
                    lst.append(t)
                xtiles[ci] = lst

            emit_x_dma(0)
            emit_x_dma(1)

            # ---- resident weights / biases (e-major so e=0 lands first) ----
            w1t, w2t, w3t, b2t, b3t = {}, {}, {}, {}, {}
            for e in range(E):
                for kc, (k0, kw) in enumerate(KC1):
                    t = wp.tile([kw, D1], MMDT, tag=f"w1_{e}_{kc}")
                    nc.sync.dma_start(t[:], w1[e, k0:k0 + kw, :])
                    w1t[e, kc] = t
                for kc, (k0, kw) in enumerate(L2K):
                    t = wp.tile([kw, D2], MMDT, tag=f"w2_{e}_{kc}")
                    nc.sync.dma_start(t[:], w2[e, k0:k0 + kw, :])
                    w2t[e, kc] = t
                for kc, (k0, kw) in enumerate(L3K):
                    t = wp.tile([kw, D3], MMDT, tag=f"w3_{e}_{kc}")
                    nc.sync.dma_start(t[:], w3[e, k0:k0 + kw, :])
                    w3t[e, kc] = t
                t = wp.tile([128, 4], F32, tag=f"b2_{e}")
                nc.sync.dma_start(t[:], b2[e])
                b2t[e] = t
                t = wp.tile([128, 4], F32, tag=f"b3_{e}")
                nc.sync.dma_start(t[:], b3[e])
                b3t[e] = t
            b1c = wp.tile([128, 1], F32, tag="b1c")
            nc.vector.memset(b1c[:], LN_A - 1.0)
            acc1t = wp.tile([128, NCOL], F32, tag="acc1")
            acc2t = wp.tile([32, NCOL], F32, tag="acc2")

            # ---- pipeline state ----
            state = {}  # it -> dict with z1, g1, z2, g2, z3 lists

            def slot_info(it):
                ci, e = divmod(it, E)
                na = min(CH, NA - ci * CH)
                return ci, e, na

            def l1_mm(it):
                ci, e, na = slot_info(it)
                if it % E == 0:
                    emit_x_dma(ci + 1)
                st = state.setdefault(it, {})
                st["z1"] = []
                for m in range(2):
                    z = ps.tile([128, CH], F32, tag="z")
                    zv = z[:, :na]
                    for kc, (k0, kw) in enumerate(KC1):
                        nc.tensor.matmul(
                            zv,
                            w1t[e, kc][:, m * 128:(m + 1) * 128],
                            xtiles[ci][kc][:kw, :na],
                            start=(kc == 0),
                            stop=(kc == len(KC1) - 1),
                        )
                    st["z1"].append(z)

            def l1_ew(it):
                ci, e, na = slot_info(it)
                st = state[it]
                st["g1"] = []
                for m in range(2):
                    zv = st["z1"][m][:, :na]
                    u = sp.tile([128, CH], F32, tag="u1")
                    nc.scalar.activation(u[:, :na], zv, AF.Exp, bias=b1c[:, 0:1], scale=10.0)
                    g = gp.tile([128, CH], MMDT, tag="g1")
                    nc.vector.scalar_tensor_tensor(g[:, :na], u[:, :na], ALPHA, zv,
                                                   op0=ALU.min, op1=ALU.max)
                    st["g1"].append(g)

            def l2_mm(it):
                ci, e, na = slot_info(it)
                st = state[it]
                st["z2"] = []
                for mi, (m0, mw) in enumerate(L2M):
                    z = ps.tile([128, CH], F32, tag="z")
                    zv = z[:mw, :na]
                    for kc, (k0, kw) in enumerate(L2K):
                        nc.tensor.matmul(
                            zv,
                            w2t[e, kc][:, m0:m0 + mw],
                            st["g1"][kc][:, :na],
                            start=(kc == 0),
                            stop=(kc == len(L2K) - 1),
                        )
                    st["z2"].append(z)

            def l2_ew(it):
                ci, e, na = slot_info(it)
                st = state[it]
                st["g2"] = []
                for mi, (m0, mw) in enumerate(L2M):
                    zv = st["z2"][mi][:mw, :na]
                    u = sp.tile([128, CH], F32, tag="u2")
                    nc.scalar.activation(u[:mw, :na], zv, AF.Exp,
                                         bias=b2t[e][:mw, 2 * mi + 1:2 * mi + 2], scale=10.0)
                    r = sp.tile([128, CH], F32, tag="r2")
                    nc.vector.tensor_scalar(r[:mw, :na], zv, b2t[e][:mw, 2 * mi:2 * mi + 1], 0.0,
                                            op0=ALU.add, op1=ALU.max)
                    g = gp.tile([128, CH], MMDT, tag="g2")
                    nc.vector.scalar_tensor_tensor(g[:mw, :na], u[:mw, :na], ALPHA, r[:mw, :na],
                                                   op0=ALU.min, op1=ALU.add)
                    st["g2"].append(g)

            def l3_mm(it):
                ci, e, na = slot_info(it)
                st = state[it]
                st["z3"] = []
                for mi, (m0, mw) in enumerate(L3M):
                    z = ps.tile([128, CH], F32, tag="z")
                    zv = z[:mw, :na]
                    for kc, (k0, kw) in enumerate(L3K):
                        nc.tensor.matmul(
                            zv,
                            w3t[e, kc][:, m0:m0 + mw],
                            st["g2"][kc][:kw, :na],
                            start=(kc == 0),
                            stop=(kc == len(L3K) - 1),
                        )
                    st["z3"].append(z)

            def l3_ew(it):
                ci, e, na = slot_info(it)
                st = state[it]
                for mi, (m0, mw) in enumerate(L3M):
                    zv = st["z3"][mi][:mw, :na]
                    accT = acc1t if mi == 0 else acc2t
                    col = (e * NCH + ci) * 2
                    # Sum relu(z+b) on ACT (accum_out = free-dim sum)
                    r = sp.tile([128, CH], F32, tag="r3")
                    nc.scalar.activation(r[:mw, :na], zv, AF.Relu,
                                         bias=b3t[e][:mw, 2 * mi:2 * mi + 1], scale=1.0,
                                         accum_out=accT[:, col:col + 1])
                    # Sum min(u', alpha) on DVE (op1=add doubles as the reduce op)
                    u = sp.tile([128, CH], F32, tag="u3")
                    nc.scalar.activation(u[:mw, :na], zv, AF.Exp,
                                         bias=b3t[e][:mw, 2 * mi + 1:2 * mi + 2], scale=10.0)
                    s2 = sp.tile([128, CH], F32, tag="s3")
                    nc.vector.tensor_scalar(s2[:mw, :na], u[:mw, :na], ALPHA, 0.0,
                                            op0=ALU.min, op1=ALU.add,
                                            accum_out=accT[:, col + 1:col + 2])
                # free pipeline state
                del state[it]

            # ---- software-pipelined main loop ----
            for t in range(NSLOT + 2):
                if t < NSLOT:
                    l1_mm(t)
                    l1_ew(t)
                if 1 <= t <= NSLOT:
                    l2_mm(t - 1)
                    l2_ew(t - 1)
                if t >= 2:
                    l3_mm(t - 2)
                    l3_ew(t - 2)

            nc.sync.dma_start(acc1[:], acc1t[:])
            nc.sync.dma_start(acc2[:], acc2t[:])
    nc.compile()
    return nc


_NC = None


def _get_nc():
    global _NC
    if _NC is None:
        _NC = _build()
    return _NC


def _prep_inputs(inputs):
    aev = np.asarray(inputs["aev"], dtype=np.float32).reshape(N, AEV)
    idx = np.asarray(inputs["idx"])
    Ws = [np.asarray(inputs[f"W{i}"], dtype=np.float32) for i in (1, 2, 3, 4)]
    bs = [np.asarray(inputs[f"b{i}"], dtype=np.float32) for i in (1, 2, 3, 4)]

    in_maps = []
    for c in range(NCORES):
        s, h = c // 2, c % 2
        sel = np.asarray(idx[s, h * NA:(h + 1) * NA])
        xTc = np.zeros((K1, NA), dtype=np.float32)
        xTc[:AEV] = aev[sel].T
        xTc[AEV] = 1.0

        # layer-1 weights with bias+alpha ones-row, zero-padded to K1
        w1c = np.zeros((E, K1, D1), dtype=np.float32)
        w1c[:, :AEV, :] = Ws[0][s]
        w1c[:, AEV, :] = bs[0][s][:, 0, :] + ALPHA
        w2c = np.ascontiguousarray(Ws[1][s])    # [8, 256, 192]
        w3c = np.zeros((E, 256, D3), dtype=np.float32)   # K zero-padded 192 -> 256
        w3c[:, :192, :] = Ws[2][s]

        b2v = bs[1][s][:, 0, :].astype(np.float64) \
            - ALPHA * Ws[1][s].astype(np.float64).sum(axis=1)            # [8, 192]
        b3v = bs[2][s][:, 0, :].astype(np.float64) \
            - ALPHA * Ws[2][s].astype(np.float64).sum(axis=1)            # [8, 160]

        def pack(bv, chunks):
            out = np.zeros((E, 128, 4), dtype=np.float32)
            for mi, (m0, mw) in enumerate(chunks):
                out[:, :mw, 2 * mi] = bv[:, m0:m0 + mw]
                out[:, :mw, 2 * mi + 1] = 10.0 * bv[:, m0:m0 + mw] + LN_A
            return out

        in_maps.append({
            "xT": xTc,
            "w1": w1c, "w2": w2c, "w3": w3c,
            "b2": pack(b2v, L2M),
            "b3": pack(b3v, L3M),
        })
    return in_maps, Ws, bs


def _finish(results, Ws, bs):
    W4 = Ws[3].astype(np.float64)  # [S, E, 160, 1]
    b4 = bs[3].astype(np.float64)  # [S, E, 1, 1]
    total = 0.0
    for c in range(NCORES):
        s = c // 2
        a1 = results[c]["acc1"].astype(np.float64)  # [128, NCOL]
        a2 = results[c]["acc2"].astype(np.float64)  # [32, NCOL]
        for e in range(E):
            cols = [(e * NCH + ci) * 2 for ci in range(NCH)]
            colsm = [cc + 1 for cc in cols]
            g3sum = np.concatenate([
                a1[:, cols].sum(axis=1) + a1[:, colsm].sum(axis=1),
                a2[:, cols].sum(axis=1) + a2[:, colsm].sum(axis=1),
            ])  # [160]
            h3sum = g3sum - ALPHA * NA
            total += (h3sum @ W4[s, e, :, 0] + NA * b4[s, e, 0, 0]) / E
    return np.array([total], dtype=np.float32)


def _run(inputs, **spmd_kwargs):
    in_maps, Ws, bs = _prep_inputs(inputs)
    nc = _get_nc()
    res = run_bass_kernel_spmd(nc, in_maps, list(range(NCORES)), **spmd_kwargs)
    return _finish(res.results, Ws, bs), res


def kernel(**inputs) -> np.ndarray:
    out, _ = _run(inputs)
    return out



# revision 12
# speedup vs baseline: 1.0513x; 1.0513x over previous
"""Bass/Trainium2 kernel for nn_BmmEnsemble (ensemble-of-MLPs atomic energy sum).

Sharding: 8 cores; core c owns species c//2, half c%2 (12500/2 = 6250 atoms).
Each core runs a 3-layer MLP (1008->256->192->160, CELU) for its species'
8 ensemble members on its 6250 atoms, producing per-feature sums of the
layer-3 activations. Layer 4, the ensemble mean, the CELU constant shifts
and the final atom sum are folded into host-side fp64 math.

v3 design (from HW microbenchmarks + trace analysis):
  - Every matmul instr costs ~226ns per 512 out-columns regardless of dtype;
    fp8 DoubleRow contracts 2 K-blocks per instr = 2x throughput.
  - L1 (76% of MACs): fp8e4m3 DoubleRow, W1 scaled x8; bias rides two spare
    K rows (r1 + r2/16, fp8) - exact to ~0.1%. 8 DR matmuls per slot.
  - L2/L3: bf16 weights (W2/8 cancels g1's 8x scale), 4+4 matmuls.
  - L3 biases ride the matmul via 2 extra K rows on the g2B chunk
    (bf16 hi + lo/256, exact to ~1e-5), so L3 elementwise ops use
    literal/const biases only.
  - PSUM allocated as [128, 1024] pair-tiles (2 banks): both m-chunks of a
    layer share one tile, so the const-bias exps (L1, L3) and the L1
    combine run as single wide ops.
  - GpSimd is avoided for bulk ops (measured ~14ns/elem: software DSP).
    ACT: L1 exp, L2 exps (AP bias), L3 exp, relu-accums rA/rB  (~3.9us/slot)
    DVE: L1 combine, m2 min, L2 combines (AP bias), sA/sB min-accums.
  Accuracy (numerically emulated): rel err ~3e-3 vs 2e-2 tolerance.
"""

import numpy as np
import ml_dtypes

import concourse.bacc as bacc
import concourse.tile as tile
import concourse.mybir as mybir
from concourse.bass_utils import run_bass_kernel_spmd

F32 = mybir.dt.float32
BF16 = mybir.dt.bfloat16
F8 = mybir.dt.float8e4
DR = mybir.MatmulPerfMode.DoubleRow
AF = mybir.ActivationFunctionType
ALU = mybir.AluOpType

NP_F8 = ml_dtypes.float8_e4m3
NP_BF = ml_dtypes.bfloat16

S = 4
E = 8
N = 50000
AEV = 1008
ALPHA = 0.1
NCORES = 8
NA = N // S // 2           # atoms per core: 6250
CH = 512                   # atom chunk (matmul free dim)
NCH = (NA + CH - 1) // CH  # 13 chunks (12 x 512 + 106)
NSLOT = E * NCH            # 104 pipeline slots
S1 = 8.0                   # L1 weight/output scale (power of 2)
LB1 = float(np.log(S1 * ALPHA) - 1.0)   # L1 exp bias const
LN_A = float(np.log(ALPHA))
D1, D2, D3 = 256, 192, 160


def _build(dbg=False):
    nc = bacc.Bacc("TRN2", target_bir_lowering=False, debug=False,
                   num_devices=NCORES)

    x8 = nc.dram_tensor("x8", [4, 128, 2, NA], F8, kind="ExternalInput")
    w1 = nc.dram_tensor("w1", [E, 4, 128, 2, D1], F8, kind="ExternalInput")
    w2 = nc.dram_tensor("w2", [E, 2, 128, D2], BF16, kind="ExternalInput")
    w3a = nc.dram_tensor("w3a", [E, 128, D3], BF16, kind="ExternalInput")
    w3b = nc.dram_tensor("w3b", [E, 66, D3], BF16, kind="ExternalInput")
    eb2 = nc.dram_tensor("eb2", [E, 128, 4], F32, kind="ExternalInput")
    accRA = nc.dram_tensor("accRA", [128, NSLOT], F32, kind="ExternalOutput")
    accMA = nc.dram_tensor("accMA", [128, NSLOT], F32, kind="ExternalOutput")
    accRB = nc.dram_tensor("accRB", [32, NSLOT], F32, kind="ExternalOutput")
    accMB = nc.dram_tensor("accMB", [32, NSLOT], F32, kind="ExternalOutput")

    with tile.TileContext(nc) as tc:
        with (
            tc.tile_pool(name="wp", bufs=1) as wp,
            tc.tile_pool(name="xp", bufs=2) as xp,
            tc.tile_pool(name="gp", bufs=3) as gp,
            tc.tile_pool(name="up", bufs=2) as up,
            tc.tile_pool(name="sp", bufs=2) as sp,
            tc.tile_pool(name="ps", bufs=4, space="PSUM") as ps,
        ):
            # ---- x prefetch ----
            xtiles = {}

            def emit_x(ci):
                if ci >= NCH or ci in xtiles:
                    return
                off = ci * CH
                na = min(CH, NA - off)
                lst = []
                for p in range(4):
                    t = xp.tile([128, 2, CH], F8, tag=f"x{p}")
                    nc.sync.dma_start(t[:, :, :na], x8[p, :, :, off:off + na])
                    lst.append(t)
                xtiles[ci] = lst

            emit_x(0)
            emit_x(1)

            # ---- resident weights / biases (e-major so e=0 lands first) ----
            w1t, w2t, w3at, w3bt, eb2t = {}, {}, {}, {}, {}
            for e in range(E):
                for p in range(4):
                    t = wp.tile([128, 2, D1], F8, tag=f"w1_{e}_{p}")
                    nc.sync.dma_start(t[:], w1[e, p])
                    w1t[e, p] = t
                for kc in range(2):
                    t = wp.tile([128, D2], BF16, tag=f"w2_{e}_{kc}")
                    nc.sync.dma_start(t[:], w2[e, kc])
                    w2t[e, kc] = t
                t = wp.tile([128, D3], BF16, tag=f"w3a_{e}")
                nc.sync.dma_start(t[:], w3a[e])
                w3at[e] = t
                t = wp.tile([66, D3], BF16, tag=f"w3b_{e}")
                nc.sync.dma_start(t[:], w3b[e])
                w3bt[e] = t
                t = wp.tile([128, 4], F32, tag=f"eb2_{e}")
                nc.sync.dma_start(t[:], eb2[e])
                eb2t[e] = t
            accRAt = wp.tile([128, NSLOT], F32, tag="accRA")
            accMAt = wp.tile([128, NSLOT], F32, tag="accMA")
            accRBt = wp.tile([32, NSLOT], F32, tag="accRB")
            accMBt = wp.tile([32, NSLOT], F32, tag="accMB")
            b1c = wp.tile([128, 1], F32, tag="b1c")
            nc.vector.memset(b1c[:], LB1)
            b3c = wp.tile([128, 1], F32, tag="b3c")
            nc.vector.memset(b3c[:], LN_A)

            state = {}

            def slot_info(it):
                ci, e = divmod(it, E)
                na = min(CH, NA - ci * CH)
                return ci, e, na

            def l1_mm(it):
                ci, e, na = slot_info(it)
                if it % E == 0:
                    emit_x(ci + 1)
                st = state.setdefault(it, {})
                z = ps.tile([128, 2 * CH], F32, tag="zp")
                for mi in range(2):
                    zv = z[:, mi * CH:mi * CH + na]
                    for p in range(4):
                        nc.tensor.matmul(
                            zv,
                            w1t[e, p][:, :, mi * 128:(mi + 1) * 128],
                            xtiles[ci][p][:, :, :na],
                            start=(p == 0), stop=(p == 3), perf_mode=DR,
                        )
                st["z1"] = z

            def l1_ew(it):
                ci, e, na = slot_info(it)
                st = state[it]
                z = st["z1"]
                w = CH + na
                u = up.tile([128, 2 * CH], BF16, tag="u1")
                nc.scalar.activation(u[:, :w], z[:, :w], AF.Exp,
                                     bias=b1c[:, 0:1],
                                     scale=float(10.0 / S1))
                g = gp.tile([128, 2 * CH], BF16, tag="g1")
                nc.vector.scalar_tensor_tensor(
                    g[:, :w], u[:, :w], float(S1 * ALPHA), z[:, :w],
                    op0=ALU.min, op1=ALU.max)
                st["g1"] = g

            def l2_mm(it):
                ci, e, na = slot_info(it)
                st = state[it]
                g1 = st["g1"]
                z = ps.tile([128, 2 * CH], F32, tag="zp")
                for mi, (m0, mw) in enumerate([(0, 128), (128, 64)]):
                    zv = z[:mw, mi * CH:mi * CH + na]
                    for kc in range(2):
                        nc.tensor.matmul(
                            zv,
                            w2t[e, kc][:, m0:m0 + mw],
                            g1[:, kc * CH:kc * CH + na],
                            start=(kc == 0), stop=(kc == 1),
                        )
                st["z2"] = z

            def l2_ew(it):
                ci, e, na = slot_info(it)
                st = state[it]
                z = st["z2"]
                u2 = up.tile([128, 2 * CH], BF16, tag="u2")
                nc.scalar.activation(u2[:, :na], z[:, :na], AF.Exp,
                                     bias=eb2t[e][:, 0:1], scale=10.0)
                nc.scalar.activation(u2[:64, CH:CH + na], z[:64, CH:CH + na],
                                     AF.Exp, bias=eb2t[e][:64, 2:3],
                                     scale=10.0)
                m2 = up.tile([128, 2 * CH], BF16, tag="m2")
                nc.vector.tensor_scalar(m2[:, :CH + na], u2[:, :CH + na],
                                        ALPHA, 0.0, op0=ALU.min, op1=ALU.add)
                gA = gp.tile([128, CH], BF16, tag="g2A")
                nc.vector.scalar_tensor_tensor(
                    gA[:, :na], z[:, :na], eb2t[e][:, 1:2], m2[:, :na],
                    op0=ALU.add, op1=ALU.max)
                gB = gp.tile([66, CH], BF16, tag="g2B")
                if it < 3:
                    nc.vector.memset(gB[64:66, :], 1.0)
                nc.vector.scalar_tensor_tensor(
                    gB[:64, :na], z[:64, CH:CH + na], eb2t[e][:64, 3:4],
                    m2[:64, CH:CH + na], op0=ALU.add, op1=ALU.max)
                st["g2"] = (gA, gB)

            def l3_mm(it):
                ci, e, na = slot_info(it)
                st = state[it]
                gA, gB = st["g2"]
                z = ps.tile([128, 2 * CH], F32, tag="zp")
                for mi, (m0, mw) in enumerate([(0, 128), (128, 32)]):
                    zv = z[:mw, mi * CH:mi * CH + na]
                    nc.tensor.matmul(zv, w3at[e][:, m0:m0 + mw], gA[:, :na],
                                     start=True, stop=False)
                    nc.tensor.matmul(zv, w3bt[e][:, m0:m0 + mw], gB[:, :na],
                                     start=False, stop=True)
                st["z3"] = z

            def l3_ew(it):
                ci, e, na = slot_info(it)
                st = state[it]
                z = st["z3"]
                u3 = up.tile([128, 2 * CH], BF16, tag="u3")
                nc.scalar.activation(u3[:, :CH + na], z[:, :CH + na], AF.Exp,
                                     bias=b3c[:, 0:1], scale=10.0)
                rA = sp.tile([128, CH], F32, tag="rA")
                nc.scalar.activation(rA[:, :na], z[:, :na], AF.Relu,
                                     accum_out=accRAt[:, it:it + 1])
                rB = sp.tile([32, CH], F32, tag="rB")
                nc.scalar.activation(rB[:, :na], z[:32, CH:CH + na], AF.Relu,
                                     accum_out=accRBt[:, it:it + 1])
                sA = sp.tile([128, CH], BF16, tag="sA")
                nc.vector.tensor_scalar(sA[:, :na], u3[:, :na], ALPHA, 0.0,
                                        op0=ALU.min, op1=ALU.add,
                                        accum_out=accMAt[:, it:it + 1])
                sB = sp.tile([32, CH], BF16, tag="sB")
                nc.vector.tensor_scalar(sB[:, :na], u3[:32, CH:CH + na],
                                        ALPHA, 0.0, op0=ALU.min, op1=ALU.add,
                                        accum_out=accMBt[:, it:it + 1])
                del state[it]

            # ---- software-pipelined main loop ----
            for t in range(NSLOT + 2):
                if t < NSLOT:
                    l1_mm(t)
                    l1_ew(t)
                if 1 <= t <= NSLOT:
                    l2_mm(t - 1)
                    l2_ew(t - 1)
                if t >= 2:
                    l3_mm(t - 2)
                    l3_ew(t - 2)

            nc.sync.dma_start(accRA[:], accRAt[:])
            nc.sync.dma_start(accMA[:], accMAt[:])
            nc.sync.dma_start(accRB[:], accRBt[:])
            nc.sync.dma_start(accMB[:], accMBt[:])
    nc.compile()
    return nc


_NC = None


def _get_nc():
    global _NC
    if _NC is None:
        _NC = _build()
    return _NC


def _prep_inputs(inputs):
    aev = np.asarray(inputs["aev"], dtype=np.float32).reshape(N, AEV)
    idx = np.asarray(inputs["idx"])
    Ws = [np.asarray(inputs[f"W{i}"], dtype=np.float32) for i in (1, 2, 3, 4)]
    bs = [np.asarray(inputs[f"b{i}"], dtype=np.float32) for i in (1, 2, 3, 4)]

    in_maps = []
    per_species = {}
    for c in range(NCORES):
        s, h = c // 2, c % 2
        if s not in per_species:
            # L1 fp8 stack with dual bias rows
            w1s = np.zeros((E, 1024, D1), dtype=np.float32)
            w1s[:, :AEV, :] = Ws[0][s] * S1
            bt = (bs[0][s][:, 0, :] + ALPHA) * S1           # [E, 256]
            r1 = bt.astype(NP_F8).astype(np.float32)
            r2 = ((bt - r1) * 16.0).astype(NP_F8).astype(np.float32)
            w1s[:, AEV, :] = r1
            w1s[:, AEV + 1, :] = r2
            w1q = w1s.astype(NP_F8)
            w1pk = np.ascontiguousarray(
                w1q.reshape(E, 4, 2, 128, D1).transpose(0, 1, 3, 2, 4))

            # L2/L3 bf16 (W2 divided by S1 to cancel g1's scale)
            w2q = (Ws[1][s] / S1).astype(NP_BF)             # [E, 256, 192]
            w3q = Ws[2][s].astype(NP_BF)                    # [E, 192, 160]
            w2pk = np.ascontiguousarray(w2q.reshape(E, 2, 128, D2))

            b2p = bs[1][s][:, 0, :].astype(np.float64) \
                - ALPHA * (w2q.astype(np.float64) * S1).sum(axis=1)  # [E,192]
            b3p = bs[2][s][:, 0, :].astype(np.float64) \
                - ALPHA * w3q.astype(np.float64).sum(axis=1)         # [E,160]

            # w3b with bias rows: row 64 = hi(b3'), row 65 = lo*256
            w3apk = np.ascontiguousarray(w3q[:, :128, :])
            w3bpk = np.zeros((E, 66, D3), dtype=NP_BF)
            w3bpk[:, :64, :] = w3q[:, 128:, :]
            b3hi = b3p.astype(NP_BF)
            b3lo = (b3p - b3hi.astype(np.float64)).astype(NP_BF)
            w3bpk[:, 64, :] = b3hi
            w3bpk[:, 65, :] = b3lo

            eb2pk = np.zeros((E, 128, 4), dtype=np.float32)
            eb2pk[:, :, 0] = 10.0 * b2p[:, :128] + LN_A
            eb2pk[:, :, 1] = b2p[:, :128] + ALPHA
            eb2pk[:, :64, 2] = 10.0 * b2p[:, 128:] + LN_A
            eb2pk[:, :64, 3] = b2p[:, 128:] + ALPHA
            per_species[s] = (w1pk, w2pk, w3apk, w3bpk, eb2pk)

        w1pk, w2pk, w3apk, w3bpk, eb2pk = per_species[s]
        sel = np.asarray(idx[s, h * NA:(h + 1) * NA])
        xTc = np.zeros((1024, NA), dtype=np.float32)
        xTc[:AEV] = aev[sel].T
        xTc[AEV] = 1.0
        xTc[AEV + 1] = 1.0 / 16.0
        x8c = np.ascontiguousarray(
            xTc.astype(NP_F8).reshape(4, 2, 128, NA).transpose(0, 2, 1, 3))

        in_maps.append({
            "x8": x8c,
            "w1": w1pk, "w2": w2pk, "w3a": w3apk, "w3b": w3bpk,
            "eb2": eb2pk,
        })
    return in_maps, Ws, bs


def _finish(results, Ws, bs):
    W4 = Ws[3].astype(np.float64)  # [S, E, 160, 1]
    b4 = bs[3].astype(np.float64)  # [S, E, 1, 1]
    total = 0.0
    for c in range(NCORES):
        s = c // 2
        rA = results[c]["accRA"].astype(np.float64)  # [128, NSLOT]
        mA = results[c]["accMA"].astype(np.float64)
        rB = results[c]["accRB"].astype(np.float64)  # [32, NSLOT]
        mB = results[c]["accMB"].astype(np.float64)
        for e in range(E):
            cols = [ci * E + e for ci in range(NCH)]
            g3sum = np.concatenate([
                rA[:, cols].sum(axis=1) + mA[:, cols].sum(axis=1),
                rB[:, cols].sum(axis=1) + mB[:, cols].sum(axis=1),
            ])  # [160]
            h3sum = g3sum - ALPHA * NA
            total += (h3sum @ W4[s, e, :, 0] + NA * b4[s, e, 0, 0]) / E
    return np.array([total], dtype=np.float32)


def _run(inputs, **spmd_kwargs):
    in_maps, Ws, bs = _prep_inputs(inputs)
    nc = _get_nc()
    res = run_bass_kernel_spmd(nc, in_maps, list(range(NCORES)), **spmd_kwargs)
    return _finish(res.results, Ws, bs), res


def kernel(**inputs) -> np.ndarray:
    out, _ = _run(inputs)
    return out


# revision 16
# speedup vs baseline: 1.3611x; 1.2946x over previous
"""Bass/Trainium2 kernel for nn_BmmEnsemble (ensemble-of-MLPs atomic energy sum).

Sharding: 8 cores; core c owns species c//2, half c%2 (12500/2 = 6250 atoms).
Each core runs a 3-layer MLP (1008->256->192->160, CELU) for its species'
8 ensemble members on its 6250 atoms, producing per-feature sums of the
layer-3 activations. Layer 4, the ensemble mean, the CELU constant shifts
and the final atom sum are folded into host-side fp64 math.

v4 design (from HW microbenchmarks + trace analysis):
  - Every matmul instr costs ~226ns per 512 out-columns regardless of dtype;
    fp8 DoubleRow contracts 2 K-blocks per instr = 2x throughput.
  - L1 (76% of MACs): fp8e4m3 DoubleRow, W1 scaled x8; bias rides two spare
    K rows (r1 + r2/16, fp8). 8 DR matmuls per slot.
  - L2/L3: bf16 weights (W2/8 cancels g1's 8x scale), 4+4 matmuls.
  - L3 biases ride the matmul via 2 extra K rows on the g2B chunk
    (bf16 hi + lo, exact to ~1e-6), so L3 elementwise ops use
    literal/const biases only.
  - The narrow L3 m1-chunk (32 features) of 4 consecutive slots is packed
    into ONE shared PSUM bank at partition offsets 0/32/64/96 (matmul
    tile_position), so its exp/relu-acc/min-acc ops run once per 4 slots
    at full 128 lanes instead of 4x at 32/128 lanes.
  - GpSimd avoided for bulk ops (software DSP, ~14ns/elem).
    ACT: exps (const/AP bias) + rA relu-accum;  DVE: combines, m2 min,
    sA/sB min-accums, rB relu-accum (literal scalars only - the
    AP-scalar+accum tensor_scalar combo mis-executes on DVE).
  Accuracy (numerically emulated): rel err ~3-6e-3 vs 2e-2 tolerance.
"""

import numpy as np
import ml_dtypes

import concourse.bacc as bacc
import concourse.tile as tile
import concourse.mybir as mybir
from concourse.bass_utils import run_bass_kernel_spmd

F32 = mybir.dt.float32
BF16 = mybir.dt.bfloat16
F8 = mybir.dt.float8e4
DR = mybir.MatmulPerfMode.DoubleRow
AF = mybir.ActivationFunctionType
ALU = mybir.AluOpType

NP_F8 = ml_dtypes.float8_e4m3
NP_BF = ml_dtypes.bfloat16

S = 4
E = 8
N = 50000
AEV = 1008
ALPHA = 0.1
NCORES = 8
NA = N // S // 2           # atoms per core: 6250
CH = 512                   # atom chunk (matmul free dim)
NCH = (NA + CH - 1) // CH  # 13 chunks (12 x 512 + 106)
NSLOT = E * NCH            # 104 pipeline slots
NGRP = NSLOT // 2          # 52 pairs of slots sharing a z3B bank
S1 = 8.0                   # L1 weight/output scale (power of 2)
LB1 = float(np.log(S1 * ALPHA) - 1.0)   # L1 exp bias const
LN_A = float(np.log(ALPHA))
D1, D2, D3 = 256, 192, 160


def _build(dbg=False):
    nc = bacc.Bacc("TRN2", target_bir_lowering=False, debug=False,
                   num_devices=NCORES)

    x8 = nc.dram_tensor("x8", [4, 128, 2, NA], F8, kind="ExternalInput")
    w1 = nc.dram_tensor("w1", [E, 4, 128, 2, D1], F8, kind="ExternalInput")
    w2 = nc.dram_tensor("w2", [E, 2, 128, D2], BF16, kind="ExternalInput")
    w3a = nc.dram_tensor("w3a", [E, 128, D3], BF16, kind="ExternalInput")
    w3b = nc.dram_tensor("w3b", [E, 66, D3], BF16, kind="ExternalInput")
    eb2 = nc.dram_tensor("eb2", [E, 128, 4], F32, kind="ExternalInput")
    accRA = nc.dram_tensor("accRA", [128, NSLOT], F32, kind="ExternalOutput")
    accMA = nc.dram_tensor("accMA", [128, NSLOT], F32, kind="ExternalOutput")
    accRB = nc.dram_tensor("accRB", [64, NGRP], F32, kind="ExternalOutput")
    accMB = nc.dram_tensor("accMB", [64, NGRP], F32, kind="ExternalOutput")

    with tile.TileContext(nc) as tc:
        with (
            tc.tile_pool(name="wp", bufs=1) as wp,
            tc.tile_pool(name="xp", bufs=2) as xp,
            tc.tile_pool(name="gp", bufs=3) as gp,
            tc.tile_pool(name="up", bufs=2) as up,
            tc.tile_pool(name="sp", bufs=2) as sp,
            tc.tile_pool(name="ps", bufs=6, space="PSUM") as ps,
            tc.tile_pool(name="psb", bufs=2, space="PSUM") as psb,
        ):
            # ---- x prefetch ----
            xtiles = {}

            def emit_x(ci):
                if ci >= NCH or ci in xtiles:
                    return
                off = ci * CH
                na = min(CH, NA - off)
                lst = []
                for p in range(4):
                    t = xp.tile([128, 2, CH], F8, tag=f"x{p}")
                    nc.sync.dma_start(t[:, :, :na], x8[p, :, :, off:off + na])
                    lst.append(t)
                xtiles[ci] = lst

            emit_x(0)
            emit_x(1)

            # ---- resident weights / biases (e-major so e=0 lands first) ----
            w1t, w2t, w3at, w3bt, eb2t = {}, {}, {}, {}, {}
            for e in range(E):
                for p in range(4):
                    t = wp.tile([128, 2, D1], F8, tag=f"w1_{e}_{p}")
                    nc.sync.dma_start(t[:], w1[e, p])
                    w1t[e, p] = t
                for kc in range(2):
                    t = wp.tile([128, D2], BF16, tag=f"w2_{e}_{kc}")
                    nc.sync.dma_start(t[:], w2[e, kc])
                    w2t[e, kc] = t
                t = wp.tile([128, D3], BF16, tag=f"w3a_{e}")
                nc.sync.dma_start(t[:], w3a[e])
                w3at[e] = t
                t = wp.tile([66, D3], BF16, tag=f"w3b_{e}")
                nc.sync.dma_start(t[:], w3b[e])
                w3bt[e] = t
                t = wp.tile([128, 4], F32, tag=f"eb2_{e}")
                nc.sync.dma_start(t[:], eb2[e])
                eb2t[e] = t
            accRAt = wp.tile([128, NSLOT], F32, tag="accRA")
            accMAt = wp.tile([128, NSLOT], F32, tag="accMA")
            accRBt = wp.tile([64, NGRP], F32, tag="accRB")
            accMBt = wp.tile([64, NGRP], F32, tag="accMB")
            b1c = wp.tile([128, 1], F32, tag="b1c")
            nc.vector.memset(b1c[:], LB1)
            b3c = wp.tile([128, 1], F32, tag="b3c")
            nc.vector.memset(b3c[:], LN_A)

            state = {}
            gstate = {}

            def slot_info(it):
                ci, e = divmod(it, E)
                na = min(CH, NA - ci * CH)
                return ci, e, na

            def l1_mm(it):
                ci, e, na = slot_info(it)
                if it % E == 0:
                    emit_x(ci + 1)
                st = state.setdefault(it, {})
                st["z1"] = []
                for mi in range(2):
                    z = ps.tile([128, CH], F32, tag="z")
                    for p in range(4):
                        nc.tensor.matmul(
                            z[:, :na],
                            w1t[e, p][:, :, mi * 128:(mi + 1) * 128],
                            xtiles[ci][p][:, :, :na],
                            start=(p == 0), stop=(p == 3), perf_mode=DR,
                        )
                    st["z1"].append(z)

            def l1_ew(it):
                ci, e, na = slot_info(it)
                st = state[it]
                st["g1"] = []
                for mi in range(2):
                    z = st["z1"][mi]
                    u = up.tile([128, CH], BF16, tag=f"u1{mi}")
                    nc.scalar.activation(u[:, :na], z[:, :na], AF.Exp,
                                         bias=b1c[:, 0:1],
                                         scale=float(10.0 / S1))
                    g = gp.tile([128, CH], BF16, tag=f"g1{mi}")
                    nc.vector.scalar_tensor_tensor(
                        g[:, :na], u[:, :na], float(S1 * ALPHA), z[:, :na],
                        op0=ALU.min, op1=ALU.max)
                    st["g1"].append(g)

            def l2_mm(it):
                ci, e, na = slot_info(it)
                st = state[it]
                g1 = st["g1"]
                zA = ps.tile([128, CH], F32, tag="z")
                zB = ps.tile([64, CH], F32, tag="z")
                for (z, m0, mw) in [(zA, 0, 128), (zB, 128, 64)]:
                    for kc in range(2):
                        nc.tensor.matmul(
                            z[:mw, :na],
                            w2t[e, kc][:, m0:m0 + mw],
                            g1[kc][:, :na],
                            start=(kc == 0), stop=(kc == 1),
                        )
                st["z2"] = (zA, zB)

            def l2_ew(it):
                ci, e, na = slot_info(it)
                st = state[it]
                zA, zB = st["z2"]
                u2 = up.tile([128, 2 * CH], BF16, tag="u2")
                nc.scalar.activation(u2[:, :na], zA[:, :na], AF.Exp,
                                     bias=eb2t[e][:, 0:1], scale=10.0)
                nc.scalar.activation(u2[:64, CH:CH + na], zB[:64, :na],
                                     AF.Exp, bias=eb2t[e][:64, 2:3],
                                     scale=10.0)
                m2 = up.tile([128, 2 * CH], BF16, tag="m2")
                nc.vector.tensor_scalar(m2[:, :CH + na], u2[:, :CH + na],
                                        ALPHA, 0.0, op0=ALU.min, op1=ALU.add)
                gA = gp.tile([128, CH], BF16, tag="g2A")
                nc.vector.scalar_tensor_tensor(
                    gA[:, :na], zA[:, :na], eb2t[e][:, 1:2], m2[:, :na],
                    op0=ALU.add, op1=ALU.max)
                gB = gp.tile([66, CH], BF16, tag="g2B")
                if it < 3:
                    nc.vector.memset(gB[64:66, :], 1.0)
                nc.vector.scalar_tensor_tensor(
                    gB[:64, :na], zB[:64, :na], eb2t[e][:64, 3:4],
                    m2[:64, CH:CH + na], op0=ALU.add, op1=ALU.max)
                st["g2"] = (gA, gB)

            def l3_mm(it):
                ci, e, na = slot_info(it)
                st = state[it]
                gA, gB = st["g2"]
                zA = ps.tile([128, CH], F32, tag="z")
                nc.tensor.matmul(zA[:, :na], w3at[e][:, 0:128], gA[:, :na],
                                 start=True, stop=False)
                nc.tensor.matmul(zA[:, :na], w3bt[e][:, 0:128], gB[:, :na],
                                 start=False, stop=True)
                k = it % 2
                if k == 0:
                    gstate[it // 2] = psb.tile([64, CH], F32, tag="zb3",
                                               name="zb3")
                zb = gstate[it // 2]
                zv = zb[32 * k:32 * k + 32, :na]
                nc.tensor.matmul(zv, w3at[e][:, 128:160], gA[:, :na],
                                 start=True, stop=False)
                nc.tensor.matmul(zv, w3bt[e][:, 128:160], gB[:, :na],
                                 start=False, stop=True)
                st["z3"] = zA

            def l3_ew(it):
                ci, e, na = slot_info(it)
                st = state[it]
                zA = st["z3"]
                u3 = up.tile([128, CH], BF16, tag="u3")
                nc.scalar.activation(u3[:, :na], zA[:, :na], AF.Exp,
                                     bias=b3c[:, 0:1], scale=10.0)
                rA = sp.tile([128, CH], F32, tag="rA")
                nc.scalar.activation(rA[:, :na], zA[:, :na], AF.Relu,
                                     accum_out=accRAt[:, it:it + 1])
                sA = sp.tile([128, CH], BF16, tag="sA")
                nc.vector.tensor_scalar(sA[:, :na], u3[:, :na], ALPHA, 0.0,
                                        op0=ALU.min, op1=ALU.add,
                                        accum_out=accMAt[:, it:it + 1])
                if it % 2 == 1:
                    j = it // 2
                    zb = gstate.pop(j)
                    uB = up.tile([64, CH], BF16, tag="u3B")
                    nc.scalar.activation(uB[:, :na], zb[:, :na], AF.Exp,
                                         bias=b3c[:64, 0:1], scale=10.0)
                    rB = sp.tile([64, CH], F32, tag="rB")
                    nc.vector.tensor_scalar(rB[:, :na], zb[:, :na], 0.0, 0.0,
                                            op0=ALU.max, op1=ALU.add,
                                            accum_out=accRBt[:, j:j + 1])
                    sB = sp.tile([64, CH], BF16, tag="sB")
                    nc.vector.tensor_scalar(sB[:, :na], uB[:, :na],
                                            ALPHA, 0.0,
                                            op0=ALU.min, op1=ALU.add,
                                            accum_out=accMBt[:, j:j + 1])
                del state[it]

            # ---- software-pipelined main loop ----
            for t in range(NSLOT + 2):
                if t < NSLOT:
                    l1_mm(t)
                    l1_ew(t)
                if 1 <= t <= NSLOT:
                    l2_mm(t - 1)
                    l2_ew(t - 1)
                if t >= 2:
                    l3_mm(t - 2)
                    l3_ew(t - 2)

            nc.sync.dma_start(accRA[:], accRAt[:])
            nc.sync.dma_start(accMA[:], accMAt[:])
            nc.sync.dma_start(accRB[:], accRBt[:])
            nc.sync.dma_start(accMB[:], accMBt[:])
    nc.compile()
    return nc


_NC = None


def _get_nc():
    global _NC
    if _NC is None:
        _NC = _build()
    return _NC


def _prep_inputs(inputs):
    aev = np.asarray(inputs["aev"], dtype=np.float32).reshape(N, AEV)
    idx = np.asarray(inputs["idx"])
    Ws = [np.asarray(inputs[f"W{i}"], dtype=np.float32) for i in (1, 2, 3, 4)]
    bs = [np.asarray(inputs[f"b{i}"], dtype=np.float32) for i in (1, 2, 3, 4)]

    in_maps = []
    per_species = {}
    for c in range(NCORES):
        s, h = c // 2, c % 2
        if s not in per_species:
            # L1 fp8 stack with dual bias rows
            w1s = np.zeros((E, 1024, D1), dtype=np.float32)
            w1s[:, :AEV, :] = Ws[0][s] * S1
            bt = (bs[0][s][:, 0, :] + ALPHA) * S1           # [E, 256]
            r1 = bt.astype(NP_F8).astype(np.float32)
            r2 = ((bt - r1) * 16.0).astype(NP_F8).astype(np.float32)
            w1s[:, AEV, :] = r1
            w1s[:, AEV + 1, :] = r2
            w1q = w1s.astype(NP_F8)
            w1pk = np.ascontiguousarray(
                w1q.reshape(E, 4, 2, 128, D1).transpose(0, 1, 3, 2, 4))

            # L2/L3 bf16 (W2 divided by S1 to cancel g1's scale)
            w2q = (Ws[1][s] / S1).astype(NP_BF)             # [E, 256, 192]
            w3q = Ws[2][s].astype(NP_BF)                    # [E, 192, 160]
            w2pk = np.ascontiguousarray(w2q.reshape(E, 2, 128, D2))

            b2p = bs[1][s][:, 0, :].astype(np.float64) \
                - ALPHA * (w2q.astype(np.float64) * S1).sum(axis=1)  # [E,192]
            b3p = bs[2][s][:, 0, :].astype(np.float64) \
                - ALPHA * w3q.astype(np.float64).sum(axis=1)         # [E,160]

            # w3b with bias rows: row 64 = hi(b3'), row 65 = lo residual
            w3apk = np.ascontiguousarray(w3q[:, :128, :])
            w3bpk = np.zeros((E, 66, D3), dtype=NP_BF)
            w3bpk[:, :64, :] = w3q[:, 128:, :]
            b3hi = b3p.astype(NP_BF)
            b3lo = (b3p - b3hi.astype(np.float64)).astype(NP_BF)
            w3bpk[:, 64, :] = b3hi
            w3bpk[:, 65, :] = b3lo

            eb2pk = np.zeros((E, 128, 4), dtype=np.float32)
            eb2pk[:, :, 0] = 10.0 * b2p[:, :128] + LN_A
            eb2pk[:, :, 1] = b2p[:, :128] + ALPHA
            eb2pk[:, :64, 2] = 10.0 * b2p[:, 128:] + LN_A
            eb2pk[:, :64, 3] = b2p[:, 128:] + ALPHA
            per_species[s] = (w1pk, w2pk, w3apk, w3bpk, eb2pk)

        w1pk, w2pk, w3apk, w3bpk, eb2pk = per_species[s]
        sel = np.asarray(idx[s, h * NA:(h + 1) * NA])
        xTc = np.zeros((1024, NA), dtype=np.float32)
        xTc[:AEV] = aev[sel].T
        xTc[AEV] = 1.0
        xTc[AEV + 1] = 1.0 / 16.0
        x8c = np.ascontiguousarray(
            xTc.astype(NP_F8).reshape(4, 2, 128, NA).transpose(0, 2, 1, 3))

        in_maps.append({
            "x8": x8c,
            "w1": w1pk, "w2": w2pk, "w3a": w3apk, "w3b": w3bpk,
            "eb2": eb2pk,
        })
    return in_maps, Ws, bs


def _finish(results, Ws, bs):
    W4 = Ws[3].astype(np.float64)  # [S, E, 160, 1]
    b4 = bs[3].astype(np.float64)  # [S, E, 1, 1]
    total = 0.0
    for c in range(NCORES):
        s = c // 2
        rA = results[c]["accRA"].astype(np.float64)  # [128, NSLOT]
        mA = results[c]["accMA"].astype(np.float64)
        rB = results[c]["accRB"].astype(np.float64)  # [64, NGRP]
        mB = results[c]["accMB"].astype(np.float64)
        for e in range(E):
            gB = np.zeros(32, dtype=np.float64)
            rBsum = np.zeros(32, dtype=np.float64)
            for ci in range(NCH):
                it = ci * E + e
                j, k = it // 2, it % 2
                rBsum += rB[32 * k:32 * k + 32, j]
                gB += mB[32 * k:32 * k + 32, j]
            cols = [ci * E + e for ci in range(NCH)]
            g3sum = np.concatenate([
                rA[:, cols].sum(axis=1) + mA[:, cols].sum(axis=1),
                rBsum + gB,
            ])  # [160]
            h3sum = g3sum - ALPHA * NA
            total += (h3sum @ W4[s, e, :, 0] + NA * b4[s, e, 0, 0]) / E
    return np.array([total], dtype=np.float32)


def _run(inputs, **spmd_kwargs):
    in_maps, Ws, bs = _prep_inputs(inputs)
    nc = _get_nc()
    res = run_bass_kernel_spmd(nc, in_maps, list(range(NCORES)), **spmd_kwargs)
    return _finish(res.results, Ws, bs), res


def kernel(**inputs) -> np.ndarray:
    out, _ = _run(inputs)
    return out
